# revision 7
# baseline (speedup 1.0000x reference)
"""Trainium2 Bass kernel for a 2-layer tanh RNN language model.

Model (see reference): x:[B,T] int indices over V=96; emb lookup -> 2 stacked
PyTorch-style tanh RNN layers (H=128) -> linear head back to V.
Returns (out [B*T, V], hidden [2, B, H]).

Strategy:
  * Data-parallel over batch: 8 cores x 32 batch rows. Params replicated.
  * Layer-0 input projection folds into a [96,128] table computed on device:
      table0 = emb @ W_ih0.T  (+ bias row), so xp0 = onehot(x) @ table0 is a
      single N=512 matmul per 16-step chunk, accumulated directly into the
      PSUM bank that the serial W_hh0 recurrence then adds onto.
  * The T=1024 scan is strictly serial; per step only:
      - 1 matmul  [128,128]x[128,32] for layer-0 (PSUM accumulate onto xp0)
      - 1 matmul for layer-1 (lagging 16 steps, PSUM accumulate onto xp1+b)
      - 1 tanh ACT over both layers' 32-col PSUM slices (3D access pattern)
  * Layer-1 input projection (W_ih1 @ h0) and the final FC run as bulk N=512
    matmuls once per chunk; biases enter via K=1 rank-1 matmuls (off the
    critical path, start=True prefills).
  * Output is produced on-chip as [96, t*32+b] per core and re-laid-out on
    host; hidden final states DMA'd as [128, 32] tiles.
"""

import sys

sys.path.insert(0, "/opt/trn_rl_repo")

import numpy as np

B, T, V, H = 256, 1024, 96, 128
NCORES = 8
BL = B // NCORES  # 32 local batch rows per core
G = 16  # time steps per chunk
NCHUNK = T // G  # 64
NW = G * BL  # 512 columns per chunk (one PSUM bank)
NT = T * BL  # 32768 columns total per core

_CACHE = {}


def _build_bass():
    import concourse.bass as bass
    import concourse.mybir as mybir
    from concourse import tile
    from concourse.bacc import Bacc

    f32 = mybir.dt.float32
    Tanh = mybir.ActivationFunctionType.Tanh

    # Bacc (not plain Bass): its finalize() runs the TRN2 legalization
    # pipeline (generate_event_semaphores splits >1-wait instructions).
    nc = Bacc()

    oh_d = nc.declare_dram_parameter("oh", [V + 1, NT], f32, isOutput=False)
    embT_d = nc.declare_dram_parameter("embT", [H, V], f32, isOutput=False)
    wih0_d = nc.declare_dram_parameter("wih0T", [H, H], f32, isOutput=False)
    whh0_d = nc.declare_dram_parameter("whh0T", [H, H], f32, isOutput=False)
    wih1_d = nc.declare_dram_parameter("wih1T", [H, H], f32, isOutput=False)
    whh1_d = nc.declare_dram_parameter("whh1T", [H, H], f32, isOutput=False)
    fcw_d = nc.declare_dram_parameter("fcWT", [H, V], f32, isOutput=False)
    bi0_d = nc.declare_dram_parameter("bih0", [1, H], f32, isOutput=False)
    bh0_d = nc.declare_dram_parameter("bhh0", [1, H], f32, isOutput=False)
    bi1_d = nc.declare_dram_parameter("bih1", [1, H], f32, isOutput=False)
    bh1_d = nc.declare_dram_parameter("bhh1", [1, H], f32, isOutput=False)
    fcb_d = nc.declare_dram_parameter("fcb", [1, V], f32, isOutput=False)

    out_d = nc.declare_dram_parameter("out", [V, NT], f32, isOutput=True)
    hT_d = nc.declare_dram_parameter("hT", [2, H, BL], f32, isOutput=True)

    with tile.TileContext(nc) as tc:
        with (
            tc.tile_pool(name="const", bufs=1) as cpool,
            tc.tile_pool(name="ohp", bufs=3) as ohp,
            tc.tile_pool(name="hp", bufs=2) as hp,
            tc.tile_pool(name="zp", bufs=2, space="PSUM") as zp,
            tc.tile_pool(name="fp", bufs=2, space="PSUM") as fp,
            tc.tile_pool(name="ip", bufs=1, space="PSUM") as ip,
            tc.tile_pool(name="op", bufs=3) as op,
        ):
            wh0 = cpool.tile([H, H], f32, name="wh0")
            nc.sync.dma_start(out=wh0, in_=whh0_d[:, :])
            wh1 = cpool.tile([H, H], f32, name="wh1")
            nc.sync.dma_start(out=wh1, in_=whh1_d[:, :])
            wi1 = cpool.tile([H, H], f32, name="wi1")
            nc.sync.dma_start(out=wi1, in_=wih1_d[:, :])
            wi0 = cpool.tile([H, H], f32, name="wi0")
            nc.sync.dma_start(out=wi0, in_=wih0_d[:, :])
            emt = cpool.tile([H, V], f32, name="emt")
            nc.sync.dma_start(out=emt, in_=embT_d[:, :])
            fcw = cpool.tile([H, V], f32, name="fcw")
            nc.sync.dma_start(out=fcw, in_=fcw_d[:, :])
            bi0 = cpool.tile([1, H], f32, name="bi0")
            nc.sync.dma_start(out=bi0, in_=bi0_d[:, :])
            bh0 = cpool.tile([1, H], f32, name="bh0")
            nc.sync.dma_start(out=bh0, in_=bh0_d[:, :])
            bi1 = cpool.tile([1, H], f32, name="bi1")
            nc.sync.dma_start(out=bi1, in_=bi1_d[:, :])
            bh1 = cpool.tile([1, H], f32, name="bh1")
            nc.sync.dma_start(out=bh1, in_=bh1_d[:, :])
            fcb = cpool.tile([1, V], f32, name="fcb")
            nc.sync.dma_start(out=fcb, in_=fcb_d[:, :])

            table = cpool.tile([V + 1, H], f32, name="table")
            b1s = cpool.tile([1, H], f32, name="b1s")
            ones = cpool.tile([1, NW], f32, name="ones")
            zro = cpool.tile([H, BL], f32, name="zro")
            nc.vector.memset(ones, 1.0)
            nc.vector.memset(zro, 0.0)

            # table0[v, o] = sum_h emb[v,h] * W_ih0[o,h]; row V = b_ih0+b_hh0
            # (tiny bf16-bitcast ldweights = "wait-carrier": walrus allows only
            # ONE sync wait on a Matmult's LDWEIGHTS struct, so pre-consume one
            # DMA dependency on the PE with a throwaway weight load.)
            bf16 = mybir.dt.bfloat16
            tps = ip.tile([V, H], f32, name="tps")
            nc.tensor.ldweights(emt[0:1, 0:1].bitcast(bf16))
            nc.tensor.matmul(tps, emt, wi0, start=True, stop=True)
            nc.vector.tensor_copy(table[0:V, :], tps)
            nc.vector.tensor_add(table[V : V + 1, :], bi0, bh0)
            nc.vector.tensor_add(b1s, bi1, bh1)

            h_prev = None
            for c in range(NCHUNK + 1):
                z = zp.tile([H, 2 * NW], f32, name="z")
                h = hp.tile([H, 2 * NW], f32, name="h")
                if c < NCHUNK:
                    oh = ohp.tile([V + 1, NW], f32, name="oh")
                    nc.sync.dma_start(out=oh, in_=oh_d[:, c * NW : (c + 1) * NW])
                    # xp0 chunk (embedding+input-proj+bias via one-hot matmul)
                    nc.tensor.ldweights(oh[0:1, 0:1].bitcast(bf16))
                    nc.tensor.matmul(z[:, 0:NW], table, oh, start=True, stop=False, skip_group_check=True)
                if c >= 1:
                    # xp1 chunk = b1 + W_ih1 @ h0[chunk c-1]
                    nc.tensor.matmul(
                        z[:, NW : 2 * NW], b1s, ones, start=True, stop=False,
                        skip_group_check=True,
                    )
                    nc.tensor.matmul(
                        z[:, NW : 2 * NW], wi1, h_prev[:, 0:NW], start=False, stop=False,
                        skip_group_check=True,
                    )
                z3 = z.rearrange("p (l n) -> p l n", l=2)
                h3 = h.rearrange("p (l n) -> p l n", l=2)
                for j in range(G):
                    s0, s1 = j * BL, (j + 1) * BL
                    if c < NCHUNK:
                        if j > 0:
                            h0p = h[:, s0 - BL : s0]
                        elif c > 0:
                            h0p = h_prev[:, NW - BL : NW]
                        else:
                            h0p = zro
                        nc.tensor.matmul(
                            z[:, s0:s1], wh0, h0p, start=False, stop=(j == G - 1),
                            skip_group_check=True,
                        )
                    if c >= 1:
                        if j > 0:
                            h1p = h[:, NW + s0 - BL : NW + s0]
                        elif c > 1:
                            h1p = h_prev[:, 2 * NW - BL : 2 * NW]
                        else:
                            h1p = zro
                        nc.tensor.matmul(
                            z[:, NW + s0 : NW + s1],
                            wh1,
                            h1p,
                            start=False,
                            stop=(j == G - 1),
                            skip_group_check=True,
                        )
                    if 1 <= c <= NCHUNK - 1:
                        nc.scalar.activation(h3[:, :, s0:s1], z3[:, :, s0:s1], Tanh)
                    elif c == 0:
                        nc.scalar.activation(h[:, s0:s1], z[:, s0:s1], Tanh)
                    else:
                        nc.scalar.activation(
                            h[:, NW + s0 : NW + s1], z[:, NW + s0 : NW + s1], Tanh
                        )
                if c >= 1:
                    # FC head over h1[chunk c-1] (cols NW:2NW of this chunk's h)
                    ft = fp.tile([V, NW], f32, name="ft")
                    ot = op.tile([V, NW], f32, name="ot")
                    if c == 1:
                        nc.tensor.ldweights(fcb[0:1, 0:1].bitcast(bf16))
                    nc.tensor.matmul(ft, fcb, ones, start=True, stop=False)
                    nc.tensor.matmul(
                        ft, fcw, h[:, NW : 2 * NW], start=False, stop=True
                    )
                    nc.vector.tensor_copy(ot, ft)
                    nc.sync.dma_start(
                        out=out_d[:, (c - 1) * NW : c * NW], in_=ot
                    )
                if c == NCHUNK - 1:
                    nc.sync.dma_start(out=hT_d[0, :, :], in_=h[:, NW - BL : NW])
                if c == NCHUNK:
                    nc.sync.dma_start(
                        out=hT_d[1, :, :], in_=h[:, 2 * NW - BL : 2 * NW]
                    )
                h_prev = h

    return nc


def _get_nc():
    if "nc" not in _CACHE:
        _CACHE["nc"] = _build_bass()
    return _CACHE["nc"]


def _make_in_maps(x, emb, W_ih0, W_hh0, b_ih0, b_hh0, W_ih1, W_hh1, b_ih1, b_hh1, fc_W, fc_b):
    f = lambda a: np.ascontiguousarray(np.asarray(a, dtype=np.float32))
    common = {
        "embT": f(np.asarray(emb).T),
        "wih0T": f(np.asarray(W_ih0).T),
        "whh0T": f(np.asarray(W_hh0).T),
        "wih1T": f(np.asarray(W_ih1).T),
        "whh1T": f(np.asarray(W_hh1).T),
        "fcWT": f(np.asarray(fc_W).T),
        "bih0": f(b_ih0).reshape(1, H),
        "bhh0": f(b_hh0).reshape(1, H),
        "bih1": f(b_ih1).reshape(1, H),
        "bhh1": f(b_hh1).reshape(1, H),
        "fcb": f(fc_b).reshape(1, V),
    }
    x = np.asarray(x)
    vr = np.arange(V)
    in_maps = []
    for i in range(NCORES):
        seq = np.ascontiguousarray(x[i * BL : (i + 1) * BL, :].T).reshape(-1)
        oh = np.empty((V + 1, NT), np.float32)
        oh[:V, :] = seq[None, :] == vr[:, None]
        oh[V, :] = 1.0
        in_maps.append({**common, "oh": oh})
    return in_maps


def _assemble(results):
    out = np.empty((B, T, V), np.float32)
    hidden = np.empty((2, B, H), np.float32)
    for i, r in enumerate(results):
        out[i * BL : (i + 1) * BL] = (
            r["out"].reshape(V, T, BL).transpose(2, 1, 0)
        )
        hidden[:, i * BL : (i + 1) * BL, :] = r["hT"].transpose(0, 2, 1)
    return out.reshape(B * T, V), hidden


def _run(in_maps, trace=False):
    from concourse.bass_utils import run_bass_kernel_spmd

    nc = _get_nc()
    if not nc.is_finalized():
        nc.finalize()
    return run_bass_kernel_spmd(
        nc, in_maps, core_ids=list(range(NCORES)), trace=trace
    )


def kernel(**inputs):
    res = _run(_make_in_maps(**inputs), trace=False)
    return _assemble(res.results)


def kernel_traced(**inputs):
    """Like kernel() but also returns the HW exec time in ns (for test.py)."""
    res = _run(_make_in_maps(**inputs), trace=True)
    return _assemble(res.results), res.exec_time_ns


# revision 16
# speedup vs baseline: 2.8082x; 2.8082x over previous
"""Trainium2 Bass kernel for a 2-layer tanh RNN language model.

Model (see reference): x:[B,T] int indices over V=96; emb lookup -> 2 stacked
PyTorch-style tanh RNN layers (H=128) -> linear head back to V.
Returns (out [B*T, V], hidden [2, B, H]).

Strategy:
  * Data-parallel over batch: 8 cores x 32 batch rows. Params replicated.
  * Layer-0 input projection folds into a [96,128] table computed on device:
      table0 = emb @ W_ih0.T  (+ bias row), so xp0 = onehot(x) @ table0 is a
      single N=512 matmul per 16-step chunk, accumulated directly into the
      PSUM bank that the serial W_hh0 recurrence then adds onto.
  * The T=1024 scan is strictly serial; per step only:
      - 1 matmul  [128,128]x[128,32] for layer-0 (PSUM accumulate onto xp0)
      - 1 matmul for layer-1 (lagging 16 steps, PSUM accumulate onto xp1+b)
      - 1 tanh ACT over both layers' 32-col PSUM slices (3D access pattern)
  * Layer-1 input projection (W_ih1 @ h0) and the final FC run as bulk N=512
    matmuls once per chunk; biases enter via K=1 rank-1 matmuls (off the
    critical path, start=True prefills).
  * Output is produced on-chip as [96, t*32+b] per core and re-laid-out on
    host; hidden final states DMA'd as [128, 32] tiles.
"""

import sys

sys.path.insert(0, "/opt/trn_rl_repo")

import numpy as np

B, T, V, H = 256, 1024, 96, 128
NCORES = 8
BL = B // NCORES  # 32 local batch rows per core
G = 16  # time steps per chunk
NCHUNK = T // G  # 64
NW = G * BL  # 512 columns per chunk (one PSUM bank)
NT = T * BL  # 32768 columns total per core

_CACHE = {}


def _build_bass():
    import concourse.bass as bass
    import concourse.mybir as mybir
    from concourse import tile
    from concourse.bacc import Bacc

    f32 = mybir.dt.float32
    f16 = mybir.dt.float16
    Tanh = mybir.ActivationFunctionType.Tanh
    Ident = mybir.ActivationFunctionType.Identity

    # Bacc (not plain Bass): its finalize() runs the TRN2 legalization
    # pipeline (generate_event_semaphores splits >1-wait instructions).
    nc = Bacc()

    oh_d = nc.declare_dram_parameter("oh", [V + 1, NT], f16, isOutput=False)
    embT_d = nc.declare_dram_parameter("embT", [H, V], f32, isOutput=False)
    wih0_d = nc.declare_dram_parameter("wih0T", [H, H], f32, isOutput=False)
    whh0_d = nc.declare_dram_parameter("whh0T", [H, H], f16, isOutput=False)
    wih1_d = nc.declare_dram_parameter("wih1T", [H, H], f16, isOutput=False)
    whh1_d = nc.declare_dram_parameter("whh1T", [H, H], f16, isOutput=False)
    fcw_d = nc.declare_dram_parameter("fcWT", [H, V], f16, isOutput=False)
    bi0_d = nc.declare_dram_parameter("bih0", [1, H], f32, isOutput=False)
    bh0_d = nc.declare_dram_parameter("bhh0", [1, H], f32, isOutput=False)
    fcb_d = nc.declare_dram_parameter("fcb", [V, 1], f32, isOutput=False)

    out_d = nc.declare_dram_parameter("out", [V, NT], f32, isOutput=True)
    hT_d = nc.declare_dram_parameter("hT", [2, H, BL], f16, isOutput=True)

    with tile.TileContext(nc) as tc:
        with (
            tc.tile_pool(name="const", bufs=1) as cpool,
            tc.tile_pool(name="ohp", bufs=3) as ohp,
            tc.tile_pool(name="hp", bufs=2) as hp,
            tc.tile_pool(name="zp", bufs=2, space="PSUM") as zp,
            tc.tile_pool(name="fp", bufs=2, space="PSUM") as fp,
            tc.tile_pool(name="ip", bufs=1, space="PSUM") as ip,
            tc.tile_pool(name="op", bufs=3) as op,
        ):
            wh0 = cpool.tile([H, H], f16, name="wh0")
            nc.sync.dma_start(out=wh0, in_=whh0_d[:, :])
            wh1 = cpool.tile([H, H], f16, name="wh1")
            nc.sync.dma_start(out=wh1, in_=whh1_d[:, :])
            wi1 = cpool.tile([H, H], f16, name="wi1")
            nc.sync.dma_start(out=wi1, in_=wih1_d[:, :])
            wi0 = cpool.tile([H, H], f32, name="wi0")
            nc.sync.dma_start(out=wi0, in_=wih0_d[:, :])
            emt = cpool.tile([H, V], f32, name="emt")
            nc.sync.dma_start(out=emt, in_=embT_d[:, :])
            fcw = cpool.tile([H, V], f16, name="fcw")
            nc.sync.dma_start(out=fcw, in_=fcw_d[:, :])
            bi0 = cpool.tile([1, H], f32, name="bi0")
            nc.sync.dma_start(out=bi0, in_=bi0_d[:, :])
            bh0 = cpool.tile([1, H], f32, name="bh0")
            nc.sync.dma_start(out=bh0, in_=bh0_d[:, :])
            fcb = cpool.tile([V, 1], f32, name="fcb")
            nc.sync.dma_start(out=fcb, in_=fcb_d[:, :])

            table = cpool.tile([V + 1, H], f16, name="table")
            b1s = cpool.tile([1, H], f16, name="b1s")
            ones = cpool.tile([1, NW], f16, name="ones")
            zro = cpool.tile([H, BL], f16, name="zro")
            nc.vector.memset(ones, 1.0)
            nc.vector.memset(zro, 0.0)

            # table0[v, o] = sum_h emb[v,h] * W_ih0[o,h]; row V = b_ih0+b_hh0
            # (tiny bf16-bitcast ldweights = "wait-carrier": walrus allows only
            # ONE sync wait on a Matmult's LDWEIGHTS struct, so pre-consume one
            # DMA dependency on the PE with a throwaway weight load.)
            bf16 = mybir.dt.bfloat16
            tps = ip.tile([V, H], f32, name="tps")
            nc.tensor.ldweights(emt[0:1, 0:1].bitcast(bf16))
            nc.tensor.matmul(tps, emt, wi0, start=True, stop=True)
            nc.vector.tensor_copy(table[0:V, :], tps)
            nc.vector.tensor_add(table[V : V + 1, :], bi0, bh0)
            nc.vector.tensor_add(b1s, bi1, bh1)

            h_prev = None
            for c in range(NCHUNK + 1):
                z = zp.tile([H, 2 * NW], f32, name="z")
                h = hp.tile([H, 2 * NW], f16, name="h")
                if c < NCHUNK:
                    oh = ohp.tile([V + 1, NW], f16, name="oh")
                    nc.sync.dma_start(out=oh, in_=oh_d[:, c * NW : (c + 1) * NW])
                    # xp0 chunk (embedding+input-proj+bias via one-hot matmul)
                    nc.tensor.ldweights(oh[0:1, 0:1].bitcast(bf16))
                    nc.tensor.matmul(z[:, 0:NW], table, oh, start=True, stop=False, skip_group_check=True)
                if c >= 1:
                    # xp1 chunk = b1 + W_ih1 @ h0[chunk c-1]
                    nc.tensor.matmul(
                        z[:, NW : 2 * NW], b1s, ones, start=True, stop=False,
                        skip_group_check=True,
                    )
                    nc.tensor.matmul(
                        z[:, NW : 2 * NW], wi1, h_prev[:, 0:NW], start=False, stop=False,
                        skip_group_check=True,
                    )
                z3 = z.rearrange("p (l n) -> p l n", l=2)
                h3 = h.rearrange("p (l n) -> p l n", l=2)
                for j in range(G):
                    s0, s1 = j * BL, (j + 1) * BL
                    if c < NCHUNK:
                        if j > 0:
                            h0p = h[:, s0 - BL : s0]
                        elif c > 0:
                            h0p = h_prev[:, NW - BL : NW]
                        else:
                            h0p = zro
                        nc.tensor.matmul(
                            z[:, s0:s1], wh0, h0p, start=False, stop=(j == G - 1),
                            skip_group_check=True,
                        )
                    if c >= 1:
                        if j > 0:
                            h1p = h[:, NW + s0 - BL : NW + s0]
                        elif c > 1:
                            h1p = h_prev[:, 2 * NW - BL : 2 * NW]
                        else:
                            h1p = zro
                        nc.tensor.matmul(
                            z[:, NW + s0 : NW + s1],
                            wh1,
                            h1p,
                            start=False,
                            stop=(j == G - 1),
                            skip_group_check=True,
                        )
                    if 1 <= c <= NCHUNK - 1:
                        nc.scalar.activation(h3[:, :, s0:s1], z3[:, :, s0:s1], Tanh)
                    elif c == 0:
                        nc.scalar.activation(h[:, s0:s1], z[:, s0:s1], Tanh)
                    else:
                        nc.scalar.activation(
                            h[:, NW + s0 : NW + s1], z[:, NW + s0 : NW + s1], Tanh
                        )
                if c >= 1:
                    # FC head over h1[chunk c-1] (cols NW:2NW of this chunk's h)
                    ft = fp.tile([V, NW], f32, name="ft")
                    ot = op.tile([V, NW], f32, name="ot")
                    nc.tensor.matmul(ft, fcw, h[:, NW : 2 * NW], start=True, stop=True)
                    nc.scalar.activation(ot, ft, Ident, bias=fcb)
                    nc.sync.dma_start(
                        out=out_d[:, (c - 1) * NW : c * NW], in_=ot
                    )
                if c == NCHUNK - 1:
                    nc.sync.dma_start(out=hT_d[0, :, :], in_=h[:, NW - BL : NW])
                if c == NCHUNK:
                    nc.sync.dma_start(
                        out=hT_d[1, :, :], in_=h[:, 2 * NW - BL : 2 * NW]
                    )
                h_prev = h

    return nc


def _get_nc():
    if "nc" not in _CACHE:
        _CACHE["nc"] = _build_bass()
    return _CACHE["nc"]


def _make_in_maps(x, emb, W_ih0, W_hh0, b_ih0, b_hh0, W_ih1, W_hh1, b_ih1, b_hh1, fc_W, fc_b):
    f = lambda a: np.ascontiguousarray(np.asarray(a, dtype=np.float32))
    g = lambda a: np.ascontiguousarray(np.asarray(a, dtype=np.float32).astype(np.float16))
    common = {
        "embT": f(np.asarray(emb).T),
        "wih0T": f(np.asarray(W_ih0).T),
        "whh0T": g(np.asarray(W_hh0).T),
        "wih1T": g(np.asarray(W_ih1).T),
        "whh1T": g(np.asarray(W_hh1).T),
        "fcWT": g(np.asarray(fc_W).T),
        "bih0": f(b_ih0).reshape(1, H),
        "bhh0": f(b_hh0).reshape(1, H),
        "fcb": f(fc_b).reshape(V, 1),
    }
    x = np.asarray(x)
    vr = np.arange(V)
    in_maps = []
    for i in range(NCORES):
        seq = np.ascontiguousarray(x[i * BL : (i + 1) * BL, :].T).reshape(-1)
        oh = np.empty((V + 1, NT), np.float16)
        oh[:V, :] = seq[None, :] == vr[:, None]
        oh[V, :] = 1.0
        in_maps.append({**common, "oh": oh})
    return in_maps


def _assemble(results):
    out = np.empty((B, T, V), np.float32)
    hidden = np.empty((2, B, H), np.float32)
    for i, r in enumerate(results):
        out[i * BL : (i + 1) * BL] = (
            r["out"].reshape(V, T, BL).transpose(2, 1, 0)
        )
        hidden[:, i * BL : (i + 1) * BL, :] = r["hT"].astype(np.float32).transpose(0, 2, 1)
    return out.reshape(B * T, V), hidden


def _run(in_maps, trace=False):
    from concourse.bass_utils import run_bass_kernel_spmd

    nc = _get_nc()
    if not nc.is_finalized():
        nc.finalize()
    return run_bass_kernel_spmd(
        nc, in_maps, core_ids=list(range(NCORES)), trace=trace
    )


def kernel(**inputs):
    res = _run(_make_in_maps(**inputs), trace=False)
    return _assemble(res.results)


def kernel_traced(**inputs):
    """Like kernel() but also returns the HW exec time in ns (for test.py)."""
    res = _run(_make_in_maps(**inputs), trace=True)
    return _assemble(res.results), res.exec_time_ns


# revision 17
# speedup vs baseline: 2.8202x; 1.0043x over previous
"""Trainium2 Bass kernel for a 2-layer tanh RNN language model.

Model (see reference): x:[B,T] int indices over V=96; emb lookup -> 2 stacked
PyTorch-style tanh RNN layers (H=128) -> linear head back to V.
Returns (out [B*T, V], hidden [2, B, H]).

Strategy:
  * Data-parallel over batch: 8 cores x 32 batch rows. Params replicated.
  * Layer-0 input projection folds into a [96,128] table computed on device:
      table0 = emb @ W_ih0.T  (+ bias row), so xp0 = onehot(x) @ table0 is a
      single N=512 matmul per 16-step chunk, accumulated directly into the
      PSUM bank that the serial W_hh0 recurrence then adds onto.
  * The T=1024 scan is strictly serial; per step only:
      - 1 matmul  [128,128]x[128,32] for layer-0 (PSUM accumulate onto xp0)
      - 1 matmul for layer-1 (lagging 16 steps, PSUM accumulate onto xp1+b)
      - 1 tanh ACT over both layers' 32-col PSUM slices (3D access pattern)
  * Layer-1 input projection (W_ih1 @ h0) and the final FC run as bulk N=512
    matmuls once per chunk; biases enter via K=1 rank-1 matmuls (off the
    critical path, start=True prefills).
  * Output is produced on-chip as [96, t*32+b] per core and re-laid-out on
    host; hidden final states DMA'd as [128, 32] tiles.
"""

import sys

sys.path.insert(0, "/opt/trn_rl_repo")

import numpy as np

B, T, V, H = 256, 1024, 96, 128
NCORES = 8
BL = B // NCORES  # 32 local batch rows per core
G = 16  # time steps per chunk
NCHUNK = T // G  # 64
NW = G * BL  # 512 columns per chunk (one PSUM bank)
NT = T * BL  # 32768 columns total per core

_CACHE = {}


def _build_bass():
    import concourse.bass as bass
    import concourse.mybir as mybir
    from concourse import tile
    from concourse.bacc import Bacc

    f32 = mybir.dt.float32
    f16 = mybir.dt.float16
    Tanh = mybir.ActivationFunctionType.Tanh
    Ident = mybir.ActivationFunctionType.Identity

    # Bacc (not plain Bass): its finalize() runs the TRN2 legalization
    # pipeline (generate_event_semaphores splits >1-wait instructions).
    nc = Bacc()

    oh_d = nc.declare_dram_parameter("oh", [V + 1, NT], f16, isOutput=False)
    embT_d = nc.declare_dram_parameter("embT", [H, V], f32, isOutput=False)
    wih0_d = nc.declare_dram_parameter("wih0T", [H, H], f32, isOutput=False)
    whh0_d = nc.declare_dram_parameter("whh0T", [H, H], f16, isOutput=False)
    wih1_d = nc.declare_dram_parameter("wih1T", [H, H], f16, isOutput=False)
    whh1_d = nc.declare_dram_parameter("whh1T", [H, H], f16, isOutput=False)
    fcw_d = nc.declare_dram_parameter("fcWT", [H, V], f16, isOutput=False)
    bi0_d = nc.declare_dram_parameter("bih0", [1, H], f32, isOutput=False)
    bh0_d = nc.declare_dram_parameter("bhh0", [1, H], f32, isOutput=False)
    fcb_d = nc.declare_dram_parameter("fcb", [V, 1], f32, isOutput=False)

    out_d = nc.declare_dram_parameter("out", [V, NT], f32, isOutput=True)
    hT_d = nc.declare_dram_parameter("hT", [2, H, BL], f16, isOutput=True)

    with tile.TileContext(nc) as tc:
        with (
            tc.tile_pool(name="const", bufs=1) as cpool,
            tc.tile_pool(name="ohp", bufs=3) as ohp,
            tc.tile_pool(name="hp", bufs=2) as hp,
            tc.tile_pool(name="zp", bufs=2, space="PSUM") as zp,
            tc.tile_pool(name="fp", bufs=2, space="PSUM") as fp,
            tc.tile_pool(name="ip", bufs=1, space="PSUM") as ip,
            tc.tile_pool(name="op", bufs=3) as op,
        ):
            wh0 = cpool.tile([H, H], f16, name="wh0")
            nc.sync.dma_start(out=wh0, in_=whh0_d[:, :])
            wh1 = cpool.tile([H, H], f16, name="wh1")
            nc.sync.dma_start(out=wh1, in_=whh1_d[:, :])
            wi1 = cpool.tile([H, H], f16, name="wi1")
            nc.sync.dma_start(out=wi1, in_=wih1_d[:, :])
            wi0 = cpool.tile([H, H], f32, name="wi0")
            nc.sync.dma_start(out=wi0, in_=wih0_d[:, :])
            emt = cpool.tile([H, V], f32, name="emt")
            nc.sync.dma_start(out=emt, in_=embT_d[:, :])
            fcw = cpool.tile([H, V], f16, name="fcw")
            nc.sync.dma_start(out=fcw, in_=fcw_d[:, :])
            bi0 = cpool.tile([1, H], f32, name="bi0")
            nc.sync.dma_start(out=bi0, in_=bi0_d[:, :])
            bh0 = cpool.tile([1, H], f32, name="bh0")
            nc.sync.dma_start(out=bh0, in_=bh0_d[:, :])
            fcb = cpool.tile([V, 1], f32, name="fcb")
            nc.sync.dma_start(out=fcb, in_=fcb_d[:, :])

            table = cpool.tile([V + 1, H], f16, name="table")
            jnk = cpool.tile([H, 256], f16, name="jnk")
            nc.vector.memset(jnk, 0.0)
            b1s = cpool.tile([1, H], f16, name="b1s")
            ones = cpool.tile([1, NW], f16, name="ones")
            zro = cpool.tile([H, BL], f16, name="zro")
            nc.vector.memset(ones, 1.0)
            nc.vector.memset(zro, 0.0)

            # table0[v, o] = sum_h emb[v,h] * W_ih0[o,h]; row V = b_ih0+b_hh0
            # (tiny bf16-bitcast ldweights = "wait-carrier": walrus allows only
            # ONE sync wait on a Matmult's LDWEIGHTS struct, so pre-consume one
            # DMA dependency on the PE with a throwaway weight load.)
            bf16 = mybir.dt.bfloat16
            tps = ip.tile([V, H], f32, name="tps")
            nc.tensor.ldweights(emt[0:1, 0:1].bitcast(bf16))
            nc.tensor.matmul(tps, emt, wi0, start=True, stop=True)
            nc.vector.tensor_copy(table[0:V, :], tps)
            nc.vector.tensor_add(table[V : V + 1, :], bi0, bh0)
            nc.vector.tensor_add(b1s, bi1, bh1)

            jps = ip.tile([H, 256], f32, name="jps", tag="tps")
            h_prev = None
            for c in range(NCHUNK + 1):
                z = zp.tile([H, 2 * NW], f32, name="z")
                h = hp.tile([H, 2 * NW], f16, name="h")
                if c < NCHUNK:
                    oh = ohp.tile([V + 1, NW], f16, name="oh")
                    nc.sync.dma_start(out=oh, in_=oh_d[:, c * NW : (c + 1) * NW])
                    # xp0 chunk (embedding+input-proj+bias via one-hot matmul)
                    nc.tensor.ldweights(oh[0:1, 0:1].bitcast(bf16))
                    nc.tensor.matmul(z[:, 0:NW], table, oh, start=True, stop=False, skip_group_check=True)
                if c >= 1:
                    # xp1 chunk = b1 + W_ih1 @ h0[chunk c-1]
                    nc.tensor.matmul(
                        z[:, NW : 2 * NW], b1s, ones, start=True, stop=False,
                        skip_group_check=True,
                    )
                    nc.tensor.matmul(
                        z[:, NW : 2 * NW], wi1, h_prev[:, 0:NW], start=False, stop=False,
                        skip_group_check=True,
                    )
                z3 = z.rearrange("p (l n) -> p l n", l=2)
                h3 = h.rearrange("p (l n) -> p l n", l=2)
                for j in range(G):
                    s0, s1 = j * BL, (j + 1) * BL
                    if c < NCHUNK:
                        if j > 0:
                            h0p = h[:, s0 - BL : s0]
                        elif c > 0:
                            h0p = h_prev[:, NW - BL : NW]
                        else:
                            h0p = zro
                        nc.tensor.matmul(
                            z[:, s0:s1], wh0, h0p, start=False, stop=(j == G - 1),
                            skip_group_check=True,
                        )
                    if c >= 1:
                        if j > 0:
                            h1p = h[:, NW + s0 - BL : NW + s0]
                        elif c > 1:
                            h1p = h_prev[:, 2 * NW - BL : 2 * NW]
                        else:
                            h1p = zro
                        nc.tensor.matmul(
                            z[:, NW + s0 : NW + s1],
                            wh1,
                            h1p,
                            start=False,
                            stop=(j == G - 1),
                            skip_group_check=True,
                        )
                    if 1 <= c <= NCHUNK - 1:
                        nc.scalar.activation(h3[:, :, s0:s1], z3[:, :, s0:s1], Tanh)
                    elif c == 0:
                        nc.scalar.activation(h[:, s0:s1], z[:, s0:s1], Tanh)
                    else:
                        nc.scalar.activation(
                            h[:, NW + s0 : NW + s1], z[:, NW + s0 : NW + s1], Tanh
                        )
                if c >= 1:
                    # FC head over h1[chunk c-1] (cols NW:2NW of this chunk's h)
                    ft = fp.tile([V, NW], f32, name="ft")
                    ot = op.tile([V, NW], f32, name="ot")
                    nc.tensor.matmul(ft, fcw, h[:, NW : 2 * NW], start=True, stop=True)
                    nc.scalar.activation(ot, ft, Ident, bias=fcb)
                    nc.sync.dma_start(
                        out=out_d[:, (c - 1) * NW : c * NW], in_=ot
                    )
                if c == NCHUNK - 1:
                    nc.sync.dma_start(out=hT_d[0, :, :], in_=h[:, NW - BL : NW])
                if c == NCHUNK:
                    nc.sync.dma_start(
                        out=hT_d[1, :, :], in_=h[:, 2 * NW - BL : 2 * NW]
                    )
                h_prev = h

    return nc


def _get_nc():
    if "nc" not in _CACHE:
        _CACHE["nc"] = _build_bass()
    return _CACHE["nc"]


def _make_in_maps(x, emb, W_ih0, W_hh0, b_ih0, b_hh0, W_ih1, W_hh1, b_ih1, b_hh1, fc_W, fc_b):
    f = lambda a: np.ascontiguousarray(np.asarray(a, dtype=np.float32))
    g = lambda a: np.ascontiguousarray(np.asarray(a, dtype=np.float32).astype(np.float16))
    common = {
        "embT": f(np.asarray(emb).T),
        "wih0T": f(np.asarray(W_ih0).T),
        "whh0T": g(np.asarray(W_hh0).T),
        "wih1T": g(np.asarray(W_ih1).T),
        "whh1T": g(np.asarray(W_hh1).T),
        "fcWT": g(np.asarray(fc_W).T),
        "bih0": f(b_ih0).reshape(1, H),
        "bhh0": f(b_hh0).reshape(1, H),
        "fcb": f(fc_b).reshape(V, 1),
    }
    x = np.asarray(x)
    vr = np.arange(V)
    in_maps = []
    for i in range(NCORES):
        seq = np.ascontiguousarray(x[i * BL : (i + 1) * BL, :].T).reshape(-1)
        oh = np.empty((V + 1, NT), np.float16)
        oh[:V, :] = seq[None, :] == vr[:, None]
        oh[V, :] = 1.0
        in_maps.append({**common, "oh": oh})
    return in_maps


def _assemble(results):
    out = np.empty((B, T, V), np.float32)
    hidden = np.empty((2, B, H), np.float32)
    for i, r in enumerate(results):
        out[i * BL : (i + 1) * BL] = (
            r["out"].reshape(V, T, BL).transpose(2, 1, 0)
        )
        hidden[:, i * BL : (i + 1) * BL, :] = r["hT"].astype(np.float32).transpose(0, 2, 1)
    return out.reshape(B * T, V), hidden


def _run(in_maps, trace=False):
    from concourse.bass_utils import run_bass_kernel_spmd

    nc = _get_nc()
    if not nc.is_finalized():
        nc.finalize()
    return run_bass_kernel_spmd(
        nc, in_maps, core_ids=list(range(NCORES)), trace=trace
    )


def kernel(**inputs):
    res = _run(_make_in_maps(**inputs), trace=False)
    return _assemble(res.results)


def kernel_traced(**inputs):
    """Like kernel() but also returns the HW exec time in ns (for test.py)."""
    res = _run(_make_in_maps(**inputs), trace=True)
    return _assemble(res.results), res.exec_time_ns


# revision 18
# speedup vs baseline: 2.8251x; 1.0017x over previous
"""Trainium2 Bass kernel for a 2-layer tanh RNN language model.

Model (see reference): x:[B,T] int indices over V=96; emb lookup -> 2 stacked
PyTorch-style tanh RNN layers (H=128) -> linear head back to V.
Returns (out [B*T, V], hidden [2, B, H]).

Strategy:
  * Data-parallel over batch: 8 cores x 32 batch rows. Params replicated.
  * Layer-0 input projection folds into a [96,128] table computed on device:
      table0 = emb @ W_ih0.T  (+ bias row), so xp0 = onehot(x) @ table0 is a
      single N=512 matmul per 16-step chunk, accumulated directly into the
      PSUM bank that the serial W_hh0 recurrence then adds onto.
  * The T=1024 scan is strictly serial; per step only:
      - 1 matmul  [128,128]x[128,32] for layer-0 (PSUM accumulate onto xp0)
      - 1 matmul for layer-1 (lagging 16 steps, PSUM accumulate onto xp1+b)
      - 1 tanh ACT over both layers' 32-col PSUM slices (3D access pattern)
  * Layer-1 input projection (W_ih1 @ h0) and the final FC run as bulk N=512
    matmuls once per chunk; biases enter via K=1 rank-1 matmuls (off the
    critical path, start=True prefills).
  * Output is produced on-chip as [96, t*32+b] per core and re-laid-out on
    host; hidden final states DMA'd as [128, 32] tiles.
"""

import sys

sys.path.insert(0, "/opt/trn_rl_repo")

import numpy as np

B, T, V, H = 256, 1024, 96, 128
NCORES = 8
BL = B // NCORES  # 32 local batch rows per core
G = 16  # time steps per chunk
NCHUNK = T // G  # 64
NW = G * BL  # 512 columns per chunk (one PSUM bank)
NT = T * BL  # 32768 columns total per core

_CACHE = {}


def _build_bass():
    import concourse.bass as bass
    import concourse.mybir as mybir
    from concourse import tile
    from concourse.bacc import Bacc

    f32 = mybir.dt.float32
    f16 = mybir.dt.float16
    Tanh = mybir.ActivationFunctionType.Tanh
    Ident = mybir.ActivationFunctionType.Identity

    # Bacc (not plain Bass): its finalize() runs the TRN2 legalization
    # pipeline (generate_event_semaphores splits >1-wait instructions).
    nc = Bacc()

    oh_d = nc.declare_dram_parameter("oh", [V + 1, NT], f16, isOutput=False)
    embT_d = nc.declare_dram_parameter("embT", [H, V], f32, isOutput=False)
    wih0_d = nc.declare_dram_parameter("wih0T", [H, H], f32, isOutput=False)
    whh0_d = nc.declare_dram_parameter("whh0T", [H, H], f16, isOutput=False)
    wih1_d = nc.declare_dram_parameter("wih1T", [H, H], f16, isOutput=False)
    whh1_d = nc.declare_dram_parameter("whh1T", [H, H], f16, isOutput=False)
    fcw_d = nc.declare_dram_parameter("fcWT", [H, V], f16, isOutput=False)
    bi0_d = nc.declare_dram_parameter("bih0", [1, H], f32, isOutput=False)
    bh0_d = nc.declare_dram_parameter("bhh0", [1, H], f32, isOutput=False)
    fcb_d = nc.declare_dram_parameter("fcb", [V, 1], f32, isOutput=False)

    out_d = nc.declare_dram_parameter("out", [V, NT], f32, isOutput=True)
    hT_d = nc.declare_dram_parameter("hT", [2, H, BL], f16, isOutput=True)

    with tile.TileContext(nc) as tc:
        with (
            tc.tile_pool(name="const", bufs=1) as cpool,
            tc.tile_pool(name="ohp", bufs=3) as ohp,
            tc.tile_pool(name="hp", bufs=2) as hp,
            tc.tile_pool(name="zp", bufs=2, space="PSUM") as zp,
            tc.tile_pool(name="fp", bufs=2, space="PSUM") as fp,
            tc.tile_pool(name="ip", bufs=1, space="PSUM") as ip,
            tc.tile_pool(name="op", bufs=3) as op,
        ):
            wh0 = cpool.tile([H, H], f16, name="wh0")
            nc.sync.dma_start(out=wh0, in_=whh0_d[:, :])
            wh1 = cpool.tile([H, H], f16, name="wh1")
            nc.sync.dma_start(out=wh1, in_=whh1_d[:, :])
            wi1 = cpool.tile([H, H], f16, name="wi1")
            nc.sync.dma_start(out=wi1, in_=wih1_d[:, :])
            wi0 = cpool.tile([H, H], f32, name="wi0")
            nc.sync.dma_start(out=wi0, in_=wih0_d[:, :])
            emt = cpool.tile([H, V], f32, name="emt")
            nc.sync.dma_start(out=emt, in_=embT_d[:, :])
            fcw = cpool.tile([H, V], f16, name="fcw")
            nc.sync.dma_start(out=fcw, in_=fcw_d[:, :])
            bi0 = cpool.tile([1, H], f32, name="bi0")
            nc.sync.dma_start(out=bi0, in_=bi0_d[:, :])
            bh0 = cpool.tile([1, H], f32, name="bh0")
            nc.sync.dma_start(out=bh0, in_=bh0_d[:, :])
            fcb = cpool.tile([V, 1], f32, name="fcb")
            nc.sync.dma_start(out=fcb, in_=fcb_d[:, :])

            table = cpool.tile([V + 1, H], f16, name="table")
            b1s = cpool.tile([1, H], f16, name="b1s")
            ones = cpool.tile([1, NW], f16, name="ones")
            zro = cpool.tile([H, BL], f16, name="zro")
            nc.vector.memset(ones, 1.0)
            nc.vector.memset(zro, 0.0)

            # table0[v, o] = sum_h emb[v,h] * W_ih0[o,h]; row V = b_ih0+b_hh0
            # (tiny bf16-bitcast ldweights = "wait-carrier": walrus allows only
            # ONE sync wait on a Matmult's LDWEIGHTS struct, so pre-consume one
            # DMA dependency on the PE with a throwaway weight load.)
            bf16 = mybir.dt.bfloat16
            tps = ip.tile([V, H], f32, name="tps")
            nc.tensor.ldweights(emt[0:1, 0:1].bitcast(bf16))
            nc.tensor.matmul(tps, emt, wi0, start=True, stop=True)
            nc.vector.tensor_copy(table[0:V, :], tps)
            nc.vector.tensor_add(table[V : V + 1, :], bi0, bh0)
            nc.vector.tensor_add(b1s, bi1, bh1)

            h_prev = None
            for c in range(NCHUNK + 1):
                z = zp.tile([H, 2 * NW], f32, name="z")
                h = hp.tile([H, 2 * NW], f16, name="h")
                if c < NCHUNK:
                    oh = ohp.tile([V + 1, NW], f16, name="oh")
                    nc.sync.dma_start(out=oh, in_=oh_d[:, c * NW : (c + 1) * NW])
                    # xp0 chunk (embedding+input-proj+bias via one-hot matmul)
                    nc.tensor.ldweights(oh[0:1, 0:1].bitcast(bf16))
                    nc.tensor.matmul(z[:, 0:NW], table, oh, start=True, stop=False, skip_group_check=True)
                if c >= 1:
                    # xp1 chunk = b1 + W_ih1 @ h0[chunk c-1]
                    nc.tensor.matmul(
                        z[:, NW : 2 * NW], b1s, ones, start=True, stop=False,
                        skip_group_check=True,
                    )
                    nc.tensor.matmul(
                        z[:, NW : 2 * NW], wi1, h_prev[:, 0:NW], start=False, stop=False,
                        skip_group_check=True,
                    )
                z3 = z.rearrange("p (l n) -> p l n", l=2)
                h3 = h.rearrange("p (l n) -> p l n", l=2)
                for j in range(G):
                    s0, s1 = j * BL, (j + 1) * BL
                    if c < NCHUNK:
                        if j > 0:
                            h0p = h[:, s0 - BL : s0]
                        elif c > 0:
                            h0p = h_prev[:, NW - BL : NW]
                        else:
                            h0p = zro
                        nc.tensor.matmul(
                            z[:, s0:s1], wh0, h0p, start=False, stop=(j == G - 1),
                            skip_group_check=True,
                        )
                    if c >= 1:
                        if j > 0:
                            h1p = h[:, NW + s0 - BL : NW + s0]
                        elif c > 1:
                            h1p = h_prev[:, 2 * NW - BL : 2 * NW]
                        else:
                            h1p = zro
                        nc.tensor.matmul(
                            z[:, NW + s0 : NW + s1],
                            wh1,
                            h1p,
                            start=False,
                            stop=(j == G - 1),
                            skip_group_check=True,
                        )
                    if 1 <= c <= NCHUNK - 1:
                        nc.scalar.activation(h3[:, :, s0:s1], z3[:, :, s0:s1], Tanh)
                    elif c == 0:
                        nc.scalar.activation(h[:, s0:s1], z[:, s0:s1], Tanh)
                    else:
                        nc.scalar.activation(
                            h[:, NW + s0 : NW + s1], z[:, NW + s0 : NW + s1], Tanh
                        )
                if c >= 1:
                    # FC head over h1[chunk c-1] (cols NW:2NW of this chunk's h)
                    ft = fp.tile([V, NW], f32, name="ft")
                    ot = op.tile([V, NW], f32, name="ot")
                    nc.tensor.matmul(ft, fcw, h[:, NW : 2 * NW], start=True, stop=True)
                    nc.scalar.activation(ot, ft, Ident, bias=fcb)
                    nc.sync.dma_start(
                        out=out_d[:, (c - 1) * NW : c * NW], in_=ot
                    )
                if c == NCHUNK - 1:
                    nc.sync.dma_start(out=hT_d[0, :, :], in_=h[:, NW - BL : NW])
                if c == NCHUNK:
                    nc.sync.dma_start(
                        out=hT_d[1, :, :], in_=h[:, 2 * NW - BL : 2 * NW]
                    )
                h_prev = h

    return nc


def _get_nc():
    if "nc" not in _CACHE:
        _CACHE["nc"] = _build_bass()
    return _CACHE["nc"]


def _make_in_maps(x, emb, W_ih0, W_hh0, b_ih0, b_hh0, W_ih1, W_hh1, b_ih1, b_hh1, fc_W, fc_b):
    f = lambda a: np.ascontiguousarray(np.asarray(a, dtype=np.float32))
    g = lambda a: np.ascontiguousarray(np.asarray(a, dtype=np.float32).astype(np.float16))
    common = {
        "embT": f(np.asarray(emb).T),
        "wih0T": f(np.asarray(W_ih0).T),
        "whh0T": g(np.asarray(W_hh0).T),
        "wih1T": g(np.asarray(W_ih1).T),
        "whh1T": g(np.asarray(W_hh1).T),
        "fcWT": g(np.asarray(fc_W).T),
        "bih0": f(b_ih0).reshape(1, H),
        "bhh0": f(b_hh0).reshape(1, H),
        "fcb": f(fc_b).reshape(V, 1),
    }
    x = np.asarray(x)
    vr = np.arange(V)
    in_maps = []
    for i in range(NCORES):
        seq = np.ascontiguousarray(x[i * BL : (i + 1) * BL, :].T).reshape(-1)
        oh = np.empty((V + 1, NT), np.float16)
        oh[:V, :] = seq[None, :] == vr[:, None]
        oh[V, :] = 1.0
        in_maps.append({**common, "oh": oh})
    return in_maps


def _assemble(results):
    out = np.empty((B, T, V), np.float32)
    hidden = np.empty((2, B, H), np.float32)
    for i, r in enumerate(results):
        out[i * BL : (i + 1) * BL] = (
            r["out"].reshape(V, T, BL).transpose(2, 1, 0)
        )
        hidden[:, i * BL : (i + 1) * BL, :] = r["hT"].astype(np.float32).transpose(0, 2, 1)
    return out.reshape(B * T, V), hidden


def _run(in_maps, trace=False):
    from concourse.bass_utils import run_bass_kernel_spmd

    nc = _get_nc()
    if not nc.is_finalized():
        nc.finalize()
    return run_bass_kernel_spmd(
        nc, in_maps, core_ids=list(range(NCORES)), trace=trace
    )


def kernel(**inputs):
    res = _run(_make_in_maps(**inputs), trace=False)
    return _assemble(res.results)


def kernel_traced(**inputs):
    """Like kernel() but also returns the HW exec time in ns (for test.py)."""
    res = _run(_make_in_maps(**inputs), trace=True)
    return _assemble(res.results), res.exec_time_ns
